# revision 42
# baseline (speedup 1.0000x reference)
"""Trainium2 Bass kernel for nn_CAGpool (GNN message passing, CAG pooling).

Sharding: data-parallel over the 64 graph pairs -> 8 pairs (16 component
graphs of 512 nodes) per NeuronCore.  Message passing is dense matmul
against a per-graph 512x512 adjacency built on-device with GPSIMD
local_scatter from host-prepared CSR index lists (index-layout prep only;
all numeric compute happens on device).

v2 restructure vs baseline:
- Both GCN norms folded into C in ONE fused DVE scalar_tensor_tensor pass
  (src norm via per-partition column, dst norm via PE-broadcast row read
  straight from PSUM).
- Attention-pool means ride free on the layer relu stores via the Act
  engine's accum_out; alpha/final pools use single-pass
  tensor_tensor_reduce instead of mult+reduce pairs.
- Pooled-conv XW matmuls + PSUM->SBUF copies are emitted before the
  serial DVE top-k loop so PE/Act overlap it; the sigmoid gate, mask and
  both pooled norms collapse into per-partition scales.
- hp kept in [feat, node] layout; dropped-node bias pollution is removed
  with exact rank-1 corrections instead of extra masking passes.
- Act engine sticks to the {copy,relu,sigmoid,tanh} and {rsqrt} tables
  only (~6 table loads total).
"""

import os
import numpy as np
import ml_dtypes

import concourse.bass as bass
import concourse.tile as tile
from concourse import bacc, mybir
from concourse.bass_utils import run_bass_kernel_spmd

F32 = mybir.dt.float32
BF16 = mybir.dt.bfloat16
I16 = mybir.dt.int16

NCORES = 8
B = 64
NPC = B // NCORES          # graph pairs per core (8)
NCG = 2 * NPC              # component graphs per core (16)
N = 512                    # nodes per component graph
K1 = 256
DEBUG = bool(int(os.environ.get("KERNEL_DEBUG", "0")))
TRACE = bool(int(os.environ.get("KERNEL_TRACE", "0")))
STAGE = int(os.environ.get("KERNEL_STAGE", "9"))
SUB = int(os.environ.get("KERNEL_SUB", "99"))


def _layout(ent):
    offs, off = {}, 0
    for nm, w in ent:
        offs[nm] = (off, w)
        off += w
    return offs, off


WOFF, WF_TOT = _layout(
    [("W1", 128), ("W2", 128), ("W3", 128), ("Wgf", 128)]
    + [(f"Wg{i}", 384) for i in range(3)]
    + [(f"Wal{i}", 768) for i in range(6)]
    + [(f"Wf{i}", 128) for i in range(3)]
    + [("Wl1a", 128), ("Wl1b", 128), ("Wl2", 64), ("Wl3", 2),
       ("cselb", 256), ("rsel", 2048)])
BOFF, BF_TOT = _layout(
    [("bcols", 3), ("bfcol", 1), ("balcol", 6), ("bl1col", 1),
     ("bl2col", 1), ("bl3col", 1), ("identf", 128)])

_GEOM = {}


def _host_prep(inputs):
    """Build per-core input maps. Index-structure prep only."""
    x = np.asarray(inputs["x"], np.float32)

    s_loc, d_loc = {}, {}
    for comp, (sk, dk) in enumerate((("src_c1", "dst_c1"),
                                     ("src_c2", "dst_c2"))):
        base = (np.arange(B) * N)[:, None]
        s_loc[comp] = np.asarray(inputs[sk]).reshape(B, -1) - base
        d_loc[comp] = np.asarray(inputs[dk]).reshape(B, -1) - base

    # Per (graph, comp): unique (s,d) pairs + multi-edge counts + degree.
    per = {}
    degs = {}
    maxw = 2
    for g in range(B):
        for comp in range(2):
            s = s_loc[comp][g]
            d = d_loc[comp][g]
            key = s.astype(np.int64) * N + d.astype(np.int64)
            key = np.concatenate([key, np.arange(N, dtype=np.int64) * (N + 1)])
            uk, cnt = np.unique(key, return_counts=True)
            us = (uk // N).astype(np.int32)
            ud = (uk % N).astype(np.int32)
            per[(g, comp)] = (us, ud, cnt.astype(np.float32))
            # degree (counts by dst, incl +1 self-loop) - integer structure
            degs[(g, comp)] = np.bincount(d, minlength=N).astype(np.float32) + 1.0
            w = np.bincount((us & 127) + 128 * (us >> 8), minlength=256).max()
            maxw = max(maxw, int(w))
    IDXW = (maxw + 1) // 2 * 2
    _GEOM["IDXW"] = IDXW

    in_maps = []
    for c in range(NCORES):
        xT = np.empty((128, NCG * N), ml_dtypes.bfloat16)
        sidx = np.full((128, NCG * 2 * IDXW), -1, np.int16)
        sdat = np.zeros((128, NCG * 2 * IDXW), ml_dtypes.bfloat16)
        degr = np.zeros((16, N), np.float32)
        for comp in range(2):
            for gl in range(NPC):
                g = c * NPC + gl
                cg = comp * NPC + gl
                r0 = g * 2 * N + comp * N
                xT[:, cg * N:(cg + 1) * N] = x[r0:r0 + N].T
                degr[cg] = degs[(g, comp)]
                us, ud, cnt = per[(g, comp)]
                sblk = us >> 7
                p = us & 127
                h = sblk >> 1
                idxval = (sblk - 2 * h) * 512 + ud
                for hh in (0, 1):
                    m = h == hh
                    pp, iv, cv = p[m], idxval[m], cnt[m]
                    order = np.argsort(pp, kind="stable")
                    pp, iv, cv = pp[order], iv[order], cv[order]
                    col = np.zeros(len(pp), np.int64)
                    _, sti, cpn = np.unique(pp, return_index=True,
                                            return_counts=True)
                    for si, cn in zip(sti, cpn):
                        col[si:si + cn] = np.arange(cn)
                    base = (cg * 2 + hh) * IDXW
                    sidx[pp, base + col] = iv.astype(np.int16)
                    sdat[pp, base + col] = cv.astype(ml_dtypes.bfloat16)

        wpack = np.zeros((128, WF_TOT), ml_dtypes.bfloat16)

        def put(nm, arr):
            o, w = WOFF[nm]
            arr = np.asarray(arr, np.float32)
            wpack[: arr.shape[0], o:o + arr.shape[1]] = arr.astype(
                ml_dtypes.bfloat16)

        put("W1", inputs["W1"]); put("W2", inputs["W2"]); put("W3", inputs["W3"])
        put("Wgf", inputs["Wg_fin"])
        for i in range(3):
            put(f"Wg{i}", np.asarray(inputs["Wg_att"])[i * 128:(i + 1) * 128])
        for i in range(6):
            put(f"Wal{i}", np.asarray(inputs["Wal"])[i * 128:(i + 1) * 128])
        for i in range(3):
            put(f"Wf{i}", np.asarray(inputs["Wf"])[i * 128:(i + 1) * 128])
        put("Wl1a", np.asarray(inputs["Wl1"])[:128])
        put("Wl1b", np.asarray(inputs["Wl1"])[128:])
        put("Wl2", inputs["Wl2"])
        put("Wl3", inputs["Wl3"])
        csel = np.zeros((128, 256), np.float32)
        for cg in range(NCG):
            csel[:, cg * 16 + cg] = 1.0
        put("cselb", csel)
        rsel = np.zeros((16, 2048), np.float32)
        for cg in range(16):
            rsel[cg, cg * 128:(cg + 1) * 128] = 1.0
        put("rsel", rsel)

        bpack = np.zeros((128, BF_TOT), np.float32)

        def putb(nm, arr):
            o, w = BOFF[nm]
            arr = np.asarray(arr, np.float32)
            bpack[: arr.shape[0], o:o + arr.shape[1]] = arr

        putb("bcols", np.stack([np.asarray(inputs["b1"]),
                                np.asarray(inputs["b2"]),
                                np.asarray(inputs["b3"])], 1))
        putb("bfcol", np.asarray(inputs["bf"])[:, None])
        putb("balcol", np.asarray(inputs["bal"]).reshape(6, 128).T)
        putb("bl1col", np.asarray(inputs["bl1"])[:, None])
        putb("bl2col", np.asarray(inputs["bl2"])[:, None])
        putb("bl3col", np.asarray(inputs["bl3"])[:, None])
        putb("identf", np.eye(128, dtype=np.float32))

        in_maps.append({"xT": np.ascontiguousarray(xT), "sidx": sidx,
                        "sdat": sdat, "degr": degr,
                        "wpack": np.ascontiguousarray(wpack), "bpack": bpack})
    return in_maps


def _build(idxw):
    nc = bacc.Bacc("TRN2", target_bir_lowering=False, debug=False,
                   num_devices=NCORES)
    tin = {
        "xT": nc.dram_tensor("xT", [128, NCG * N], BF16, kind="ExternalInput"),
        "sidx": nc.dram_tensor("sidx", [128, NCG * 2 * idxw], I16,
                               kind="ExternalInput"),
        "sdat": nc.dram_tensor("sdat", [128, NCG * 2 * idxw], BF16,
                               kind="ExternalInput"),
        "degr": nc.dram_tensor("degr", [16, N], F32, kind="ExternalInput"),
        "wpack": nc.dram_tensor("wpack", [128, WF_TOT], BF16,
                                kind="ExternalInput"),
        "bpack": nc.dram_tensor("bpack", [128, BF_TOT], F32,
                                kind="ExternalInput"),
    }
    t_out = nc.dram_tensor("out", [2, NPC], F32, kind="ExternalOutput")
    dbg = {}
    if DEBUG:
        for nm, shape, dt in (
                ("C", [128, NCG * 2048], BF16), ("deg", [16, N], F32),
                ("xcatT", [128, NCG * 1536], BF16), ("pvT", [128, 48], F32),
                ("scores", [16, N], F32), ("mask", [16, N], F32),
                ("meanT", [128, 48], F32), ("gpT", [128, 48], F32),
                ("deg2", [16, N], F32), ("qrow", [16, N], F32),
                ("hp", [128, NCG * 512], BF16), ("gT", [128, 16], F32),
                ("mean2", [128, 16], F32), ("xwps", [128, NCG * 512], BF16)):
            dbg[nm] = nc.dram_tensor("dbg_" + nm, shape, dt,
                                     kind="ExternalOutput")
    with tile.TileContext(nc, linearize=bool(int(os.environ.get(
            "KERNEL_LINEARIZE", "0")))) as tc:
        _emit(nc, tc, tin, t_out, idxw, dbg)
    nc.compile()
    return nc


def _emit(nc, tc, tin, t_out, idxw, dbg):
    import contextlib
    ctx = contextlib.ExitStack()
    AX = mybir.AxisListType.X
    OP = mybir.AluOpType
    ACT = mybir.ActivationFunctionType

    const = ctx.enter_context(tc.tile_pool(name="const", bufs=1))
    rows = ctx.enter_context(tc.tile_pool(name="rows", bufs=1))
    work = ctx.enter_context(tc.tile_pool(name="work", bufs=2))
    hwork = ctx.enter_context(tc.tile_pool(name="hwork", bufs=2))
    ps_mm = ctx.enter_context(tc.tile_pool(name="ps_mm", bufs=2, space="PSUM"))
    ps_ag = ctx.enter_context(tc.tile_pool(name="ps_ag", bufs=2, space="PSUM"))
    ps_bc = ctx.enter_context(tc.tile_pool(name="ps_bc", bufs=2, space="PSUM"))
    ps_st = ctx.enter_context(tc.tile_pool(name="ps_st", bufs=1, space="PSUM"))
    ps_t = ctx.enter_context(tc.tile_pool(name="ps_t", bufs=1, space="PSUM"))

    wb = const.tile([128, WF_TOT], BF16, tag="wb")
    bp = const.tile([128, BF_TOT], F32, tag="bp")
    xTb = const.tile([128, NCG * N], BF16, tag="xTb")   # reused as hp later
    Call = const.tile([128, NCG * 2048], BF16, tag="Call")
    xcatT = const.tile([128, NCG * 1536], BF16, tag="xcatT")
    rsdcol = const.tile([128, 64], F32, tag="rsdcol")
    msqcol = const.tile([128, 64], F32, tag="msqcol")
    gqcol = const.tile([128, 64], F32, tag="gqcol")
    meanT = const.tile([128, 48], F32, tag="meanT")
    gpT = const.tile([128, 48], F32, tag="gpT")
    cTf = const.tile([128, 48], F32, tag="cTf")
    hsum = const.tile([128, 16], F32, tag="hsum")
    gT = const.tile([128, 16], F32, tag="gT")

    def W(nm):
        o, w = WOFF[nm]
        return wb[:, o:o + w]

    def Bc(nm):
        o, w = BOFF[nm]
        return bp[:, o:o + w]

    nc.sync.dma_start(wb[:], tin["wpack"].ap())
    nc.sync.dma_start(bp[:], tin["bpack"].ap())
    nc.sync.dma_start(xTb[:], tin["xT"].ap())

    identf = Bc("identf")

    def cselb(cg):
        o, _ = WOFF["cselb"]
        return wb[:, o + cg * 16: o + (cg + 1) * 16]

    def rself(cg):
        o, _ = WOFF["rsel"]
        return wb[0:16, o + cg * 128: o + (cg + 1) * 128]

    def bcast_row(row_tile, cg, n):
        """Broadcast row cg of a [16, n] bf16 tile to [128, n] PSUM f32."""
        pb = ps_bc.tile([128, 512], F32, tag="bcast")
        nc.tensor.matmul(pb[:, :n], lhsT=rself(cg), rhs=row_tile[0:16, 0:n],
                         start=True, stop=True)
        return pb

    def t16(row_tile, sblk, out_col, out_tile):
        """Transpose [16,128] f32 row chunk sblk into out_tile[:, 16-col]."""
        pt = ps_t.tile([128, 128], F32, tag="tp")
        nc.tensor.transpose(pt[:, 0:16],
                            row_tile[:, sblk * 128:(sblk + 1) * 128],
                            identf[0:16, 0:16])
        nc.vector.tensor_copy(out_tile[:, out_col:out_col + 16], pt[:, 0:16])

    # ---- degree norm rows (host-shipped integer degree) -------------------
    deg_row = rows.tile([16, N], F32, tag="deg")
    nc.sync.dma_start(deg_row[:], tin["degr"].ap())
    sq1_row = rows.tile([16, N], F32, tag="sq1")
    nc.scalar.activation(sq1_row[:], deg_row[:], ACT.Sqrt)    # table A
    rsd_row = rows.tile([16, N], F32, tag="rsd")
    nc.vector.reciprocal(rsd_row[:], sq1_row[:])
    rsd_rowb = rows.tile([16, N], BF16, tag="rsdb")
    nc.vector.tensor_copy(rsd_rowb[:], rsd_row[:])
    for sblk in range(4):
        t16(rsd_row, sblk, sblk * 16, rsdcol)
    if DEBUG:
        nc.sync.dma_start(dbg["deg"].ap(), deg_row[:])

    # ---- build C (scatter raw counts incl +I) -----------------------------
    with tc.tile_pool(name="edges", bufs=1) as epool:
        sidx = epool.tile([128, NCG * 2 * idxw], I16, tag="sidx")
        sdat = epool.tile([128, NCG * 2 * idxw], BF16, tag="sdat")
        nc.sync.dma_start(sidx[:], tin["sidx"].ap())
        nc.sync.dma_start(sdat[:], tin["sdat"].ap())
        for cg in range(NCG):
            for h in (0, 1):
                b0 = (cg * 2 + h) * idxw
                nc.gpsimd.local_scatter(
                    Call[:, cg * 2048 + h * 1024: cg * 2048 + (h + 1) * 1024],
                    sdat[:, b0:b0 + idxw], sidx[:, b0:b0 + idxw],
                    channels=128, num_elems=1024, num_idxs=idxw)

    # ---- fold BOTH norms into C: Chat = rsd[s] * rsd[d] * C ---------------
    # dst side: 2x-mode DVE multiply against an SBUF bf16 broadcast;
    # src side: in-place Act per-partition scale.
    for cg in range(NCG):
        pb = bcast_row(rsd_rowb, cg, N)
        pabf = work.tile([128, 512], BF16, tag="scr")
        nc.scalar.activation(pabf[:], pb[:], ACT.Copy)
        for sblk in range(4):
            sl = Call[:, cg * 2048 + sblk * 512: cg * 2048 + (sblk + 1) * 512]
            nc.vector.tensor_tensor(sl, sl, pabf[:], op=OP.mult)
            nc.scalar.activation(
                sl, sl, ACT.Copy,
                scale=rsdcol[:, sblk * 16 + cg: sblk * 16 + cg + 1])
    if DEBUG:
        nc.sync.dma_start(dbg["C"].ap(), Call[:])

    # ---- 3 GCN layers (mean accumulates free via Act accum_out) -----------
    if STAGE < 2:
        _finish_dummy(nc, rows, t_out, ctx)
        return
    for l in range(3):
        wl = W(("W1", "W2", "W3")[l])
        bcol = Bc("bcols")[:, l:l + 1]
        for cg in range(NCG):
            pxw = ps_mm.tile([128, 512], F32, tag="mmw")
            for nt in range(4):
                if l == 0:
                    lhsT = xTb[:, cg * N + nt * 128: cg * N + (nt + 1) * 128]
                else:
                    lhsT = xcatT[:, cg * 1536 + (l - 1) * 512 + nt * 128:
                                 cg * 1536 + (l - 1) * 512 + (nt + 1) * 128]
                nc.tensor.matmul(pxw[:, nt * 128:(nt + 1) * 128], lhsT=lhsT,
                                 rhs=wl, start=True, stop=True)
            xws = work.tile([128, 512], BF16, tag="xws")
            if l == 1:
                nc.vector.tensor_copy(xws[:], pxw[:])
            else:
                nc.scalar.activation(xws[:], pxw[:], ACT.Copy)
            ph = ps_ag.tile([128, 512], F32, tag="agg")
            for sblk in range(4):
                nc.tensor.matmul(
                    ph[:],
                    lhsT=xws[:, sblk * 128:(sblk + 1) * 128],
                    rhs=Call[:, cg * 2048 + sblk * 512:
                             cg * 2048 + (sblk + 1) * 512],
                    start=(sblk == 0), stop=(sblk == 3))
            nc.scalar.activation(
                xcatT[:, cg * 1536 + l * 512: cg * 1536 + (l + 1) * 512],
                ph[:], ACT.Relu, bias=bcol,
                accum_out=meanT[:, l * 16 + cg: l * 16 + cg + 1])
    if DEBUG:
        nc.sync.dma_start(dbg["xcatT"].ap(), xcatT[:])
        nc.sync.dma_start(dbg["meanT"].ap(), meanT[:])

    # ---- attention pool: c = tanh(mean @ Wg) ------------------------------
    if STAGE < 3:
        _finish_dummy(nc, rows, t_out, ctx)
        return
    meanTb = rows.tile([128, 48], BF16, tag="meanTb")
    nc.scalar.activation(meanTb[:], meanT[:], ACT.Copy, scale=1.0 / N)
    for fo in range(3):
        pc = ps_t.tile([128, 128], F32, tag="tp")
        for fi in range(3):
            nc.tensor.matmul(pc[:, 0:16],
                             lhsT=W(f"Wg{fi}")[:, fo * 128:(fo + 1) * 128],
                             rhs=meanTb[:, fi * 16:(fi + 1) * 16],
                             start=(fi == 0), stop=(fi == 2))
        nc.scalar.activation(cTf[:, fo * 16:(fo + 1) * 16], pc[:, 0:16],
                             ACT.Tanh)                     # table B
    if SUB < 2:
        _finish_dummy(nc, rows, t_out, ctx)
        return

    # ---- alpha logits: one [16,512] PSUM pass over xcat -------------------
    ps_al = ps_st.tile([16, N], F32, tag="stat")
    for cg in range(NCG):
        for ch in range(3):
            mlh = work.tile([128, 16], BF16, tag="mlh")
            nc.vector.tensor_scalar(
                mlh[:], cselb(cg), cTf[:, ch * 16 + cg: ch * 16 + cg + 1],
                None, op0=OP.mult)
            nc.tensor.matmul(
                ps_al[:], lhsT=mlh[:],
                rhs=xcatT[:, cg * 1536 + ch * 512: cg * 1536 + (ch + 1) * 512],
                start=(cg == 0 and ch == 0),
                stop=(cg == NCG - 1 and ch == 2))
    alpha_rowb = rows.tile([16, N], BF16, tag="alpha")
    nc.scalar.activation(alpha_rowb[:], ps_al[:], ACT.Sigmoid)
    if SUB < 3:
        _finish_dummy(nc, rows, t_out, ctx)
        return

    # ---- gp = sum_n alpha[n] * xcat[n] via fused TTR ----------------------
    for cg in range(NCG):
        pab = bcast_row(alpha_rowb, cg, N)
        for ch in range(3):
            scr = work.tile([128, 512], BF16, tag="scr")
            nc.vector.scalar_tensor_tensor(
                scr[:],
                xcatT[:, cg * 1536 + ch * 512: cg * 1536 + (ch + 1) * 512],
                1.0, pab[:], op0=OP.mult, op1=OP.mult,
                accum_out=gpT[:, ch * 16 + cg: ch * 16 + cg + 1])
    if DEBUG:
        nc.sync.dma_start(dbg["gpT"].ap(), gpT[:])

    # ---- pv = att_lin(concat pools) ---------------------------------------
    if STAGE < 4:
        _finish_dummy(nc, rows, t_out, ctx)
        return
    gpcatTb = rows.tile([128, 48], BF16, tag="gpcatTb")
    for j in range(6):
        comp, ch = j // 3, j % 3
        nc.vector.tensor_copy(
            gpcatTb[:, j * 8:(j + 1) * 8],
            gpT[:, ch * 16 + comp * 8: ch * 16 + comp * 8 + 8])
    pvTb = rows.tile([128, 48], BF16, tag="pvTb")
    pvTf = rows.tile([128, 48], F32, tag="pvTf")
    for co in range(6):
        pp = ps_t.tile([128, 128], F32, tag="tp")
        for ci in range(6):
            nc.tensor.matmul(pp[:, 0:8],
                             lhsT=W(f"Wal{ci}")[:, co * 128:(co + 1) * 128],
                             rhs=gpcatTb[:, ci * 8:(ci + 1) * 8],
                             start=(ci == 0), stop=(ci == 5))
        nc.vector.tensor_scalar(pvTf[:, co * 8:(co + 1) * 8], pp[:, 0:8],
                                Bc("balcol")[:, co:co + 1], None, op0=OP.add)
        nc.vector.tensor_copy(pvTb[:, co * 8:(co + 1) * 8],
                              pvTf[:, co * 8:(co + 1) * 8])
    if DEBUG:
        nc.sync.dma_start(dbg["pvT"].ap(), pvTf[:])

    # ---- 1/||pv|| ----------------------------------------------------------
    pn = ps_t.tile([128, 128], F32, tag="tp")
    for ci in range(6):
        comp = ci // 3
        mpv = work.tile([128, 16], BF16, tag="mlh")
        nc.vector.memset(mpv[:], 0.0)
        nc.vector.tensor_copy(mpv[:, comp * 8:(comp + 1) * 8],
                              pvTb[:, ci * 8:(ci + 1) * 8])
        nc.tensor.matmul(pn[0:16, 0:16], lhsT=mpv[:], rhs=mpv[:],
                         start=(ci == 0), stop=(ci == 5))
    dd = work.tile([16, 16], F32, tag="dd")
    nc.vector.tensor_tensor(dd[:], pn[0:16, 0:16], identf[0:16, 0:16],
                            op=OP.mult)
    nn = rows.tile([16, 1], F32, tag="nn")
    nc.vector.tensor_reduce(nn[:], dd[:], axis=AX, op=OP.add)
    # ---- scores (raw, unnormalized: topk order is scale-invariant) --------
    ps_sc = ps_st.tile([16, N], F32, tag="stat")
    for cg in range(NCG):
        comp, g = cg // NPC, cg % NPC
        for ci in range(3):
            mlh = work.tile([128, 16], BF16, tag="mlh")
            nc.vector.tensor_scalar(
                mlh[:], cselb(cg),
                pvTf[:, (comp * 3 + ci) * 8 + g:(comp * 3 + ci) * 8 + g + 1],
                None, op0=OP.mult)
            nc.tensor.matmul(
                ps_sc[:], lhsT=mlh[:],
                rhs=xcatT[:, cg * 1536 + ci * 512: cg * 1536 + (ci + 1) * 512],
                start=(cg == 0 and ci == 0),
                stop=(cg == NCG - 1 and ci == 2))
    score_row = rows.tile([16, N], F32, tag="score")
    nc.scalar.activation(score_row[:], ps_sc[:], ACT.Copy)

    # ---- top-256 on raw scores, XWp matmuls interleaved for PE overlap ----
    if STAGE < 5:
        _finish_dummy(nc, rows, t_out, ctx)
        return
    # allocated here so it reuses the SBUF freed by the edges pool
    xwpool = ctx.enter_context(tc.tile_pool(name="xwpool", bufs=1))
    xwps = xwpool.tile([128, NCG * 512], BF16, tag="xwps")

    def emit_xwp(cg):
        pxp = ps_mm.tile([128, 512], F32, tag="mmw")
        for nt in range(4):
            for ci in range(3):
                nc.tensor.matmul(
                    pxp[:, nt * 128:(nt + 1) * 128],
                    lhsT=xcatT[:, cg * 1536 + ci * 512 + nt * 128:
                               cg * 1536 + ci * 512 + (nt + 1) * 128],
                    rhs=W(f"Wf{ci}"), start=(ci == 0), stop=(ci == 2))
        nc.scalar.activation(xwps[:, cg * 512:(cg + 1) * 512], pxp[:],
                             ACT.Copy)

    cur = rows.tile([16, N], F32, tag="cur")
    nc.vector.tensor_copy(cur[:], score_row[:])
    mx = rows.tile([16, 8], F32, tag="mx")
    for r in range(K1 // 8):
        nc.vector.max(out=mx[:], in_=cur[:])
        nc.vector.match_replace(out=cur[:], in_to_replace=mx[:],
                                in_values=cur[:], imm_value=-1e30)
        if r % 2 == 0 and r // 2 < NCG:
            emit_xwp(r // 2)
    # 1/||pv|| computed under the topk window (Act idle there)
    sqn = rows.tile([16, 1], F32, tag="sqn")
    nc.scalar.activation(sqn[:], nn[:], ACT.Sqrt)              # table A
    rsncol = rows.tile([16, 1], F32, tag="rsncol")
    nc.vector.reciprocal(rsncol[:], sqn[:])
    mask_row = rows.tile([16, N], F32, tag="mask")
    nc.vector.tensor_tensor(mask_row[:], score_row[:], cur[:], op=OP.not_equal)
    sig_row = rows.tile([16, N], F32, tag="sig")
    nc.scalar.activation(sig_row[:], score_row[:], ACT.Sigmoid,
                         scale=rsncol[:])                      # table B
    if DEBUG:
        nc.sync.dma_start(dbg["scores"].ap(), score_row[:])
        nc.sync.dma_start(dbg["mask"].ap(), mask_row[:])
    if STAGE < 6:
        _finish_dummy(nc, rows, t_out, ctx)
        return

    # ---- pooled degree: one [16,512] pass over Chat -----------------------
    msq_row = rows.tile([16, N], F32, tag="msq")
    nc.vector.tensor_tensor(msq_row[:], mask_row[:], sq1_row[:], op=OP.mult)
    for sblk in range(4):
        t16(msq_row, sblk, sblk * 16, msqcol)
    ps_d2 = ps_st.tile([16, N], F32, tag="stat")
    for cg in range(NCG):
        for sblk in range(4):
            mlh = work.tile([128, 16], BF16, tag="mlh")
            nc.vector.tensor_scalar(
                mlh[:], cselb(cg),
                msqcol[:, sblk * 16 + cg: sblk * 16 + cg + 1], None,
                op0=OP.mult)
            nc.tensor.matmul(
                ps_d2[:], lhsT=mlh[:],
                rhs=Call[:, cg * 2048 + sblk * 512: cg * 2048 + (sblk + 1) * 512],
                start=(cg == 0 and sblk == 0),
                stop=(cg == NCG - 1 and sblk == 3))
    deg2_row = rows.tile([16, N], F32, tag="deg2")
    nc.vector.tensor_tensor(deg2_row[:], ps_d2[:], sq1_row[:], op=OP.mult)
    nc.vector.scalar_tensor_tensor(deg2_row[:], deg2_row[:], 1.0, mask_row[:],
                                   op0=OP.add, op1=OP.subtract)
    if DEBUG:
        nc.sync.dma_start(dbg["deg2"].ap(), deg2_row[:])
    nc.scalar.activation(deg2_row[:], deg2_row[:], ACT.Sqrt)    # table A
    rsd2_row = rows.tile([16, N], F32, tag="rsd2")
    nc.vector.reciprocal(rsd2_row[:], deg2_row[:])
    q_row = rows.tile([16, N], F32, tag="qrow")
    nc.vector.tensor_tensor(q_row[:], rsd2_row[:], msq_row[:], op=OP.mult)
    qrowb = rows.tile([16, N], BF16, tag="qrowb")
    nc.vector.tensor_copy(qrowb[:], q_row[:])
    gq_row = sig_row  # sigmoid row dead after this product
    nc.vector.tensor_tensor(gq_row[:], sig_row[:], q_row[:], op=OP.mult)
    for sblk in range(4):
        t16(gq_row, sblk, sblk * 16, gqcol)
    if DEBUG:
        nc.sync.dma_start(dbg["qrow"].ap(), q_row[:])

    # ---- pooled conv: scale xwps by gate*rsd2*sq1, agg, relu --------------
    hpall = xTb  # xTb fully consumed by layer 1
    bfcol = Bc("bfcol")[:, 0:1]
    for cg in range(NCG):
        for nt in range(4):
            sl = xwps[:, cg * 512 + nt * 128: cg * 512 + (nt + 1) * 128]
            nc.scalar.activation(sl, sl, ACT.Copy,
                                 scale=gqcol[:, nt * 16 + cg: nt * 16 + cg + 1])
        pm = ps_ag.tile([128, 512], F32, tag="agg")
        for sblk in range(4):
            nc.tensor.matmul(
                pm[:],
                lhsT=xwps[:, cg * 512 + sblk * 128: cg * 512 + (sblk + 1) * 128],
                rhs=Call[:, cg * 2048 + sblk * 512: cg * 2048 + (sblk + 1) * 512],
                start=(sblk == 0), stop=(sblk == 3))
        pbq = bcast_row(qrowb, cg, N)
        qb = work.tile([128, 512], BF16, tag="scr")
        nc.scalar.activation(qb[:], pbq[:], ACT.Copy)
        hpre = hwork.tile([128, 512], F32, tag="hpre")
        nc.vector.tensor_tensor(hpre[:], pm[:], qb[:], op=OP.mult)
        nc.scalar.activation(hpall[:, cg * 512:(cg + 1) * 512], hpre[:],
                             ACT.Relu, bias=bfcol,
                             accum_out=hsum[:, cg:cg + 1])
    if DEBUG:
        nc.sync.dma_start(dbg["hp"].ap(), hpall[:])
        nc.sync.dma_start(dbg["xwps"].ap(), xwps[:])
    if STAGE < 7:
        _finish_dummy(nc, rows, t_out, ctx)
        return

    # ---- final attention pool with exact dropped-column corrections -------
    relu1 = rows.tile([128, 1], F32, tag="relu1")
    nc.scalar.activation(relu1[:], bfcol, ACT.Relu)
    relu256 = rows.tile([128, 1], F32, tag="relu256")
    nc.scalar.activation(relu256[:], bfcol, ACT.Relu, scale=float(N - K1))
    mean2 = rows.tile([128, 16], F32, tag="mean2")
    for cg in range(NCG):
        nc.vector.scalar_tensor_tensor(
            mean2[:, cg:cg + 1], hsum[:, cg:cg + 1], 1.0 / K1, relu1[:],
            op0=OP.mult, op1=OP.subtract)
    # mean2 = hsum/K - (N-K)/K * relu(bf); (N-K)/K = 1 here since K = N/2
    mean2b = rows.tile([128, 16], BF16, tag="mean2b")
    nc.vector.tensor_copy(mean2b[:], mean2[:])
    if DEBUG:
        nc.sync.dma_start(dbg["mean2"].ap(), mean2[:])
    pc2 = ps_t.tile([128, 128], F32, tag="tp")
    nc.tensor.matmul(pc2[:, 0:16], lhsT=W("Wgf"), rhs=mean2b[:], start=True,
                     stop=True)
    c2f = rows.tile([128, 16], F32, tag="c2f")
    nc.scalar.activation(c2f[:], pc2[:, 0:16], ACT.Tanh)        # table B
    c2fb = rows.tile([128, 16], BF16, tag="c2fb")
    nc.vector.tensor_copy(c2fb[:], c2f[:])

    # alpha2 in two half-groups so the g-pool overlaps the second group.
    # Both halves route into PSUM partitions 0-7 (engines cannot read PSUM
    # at base partition 8): selector cols [cg*16+half*8 : +8] put row cg%8.
    ocs, _ = WOFF["cselb"]
    ors, _ = WOFF["rsel"]
    for half in range(2):
        a2h = rows.tile([8, N], BF16, tag=f"a2row{half}")
        ps_a2 = ps_st.tile([16, N], F32, tag="stat")
        for cg in range(half * 8, half * 8 + 8):
            c2m = work.tile([128, 8], BF16, tag="mlh8")
            nc.vector.tensor_scalar(
                c2m[:], wb[:, ocs + cg * 16 + half * 8: ocs + cg * 16 + half * 8 + 8],
                c2f[:, cg:cg + 1], None, op0=OP.mult)
            nc.tensor.matmul(ps_a2[0:8, :], lhsT=c2m[:],
                             rhs=hpall[:, cg * 512:(cg + 1) * 512],
                             start=(cg == half * 8), stop=(cg == half * 8 + 7))
        nc.scalar.activation(a2h[:], ps_a2[0:8, :], ACT.Sigmoid)
        for cg in range(half * 8, half * 8 + 8):
            r = cg % 8
            pba = ps_bc.tile([128, 512], F32, tag="bcast")
            nc.tensor.matmul(pba[:], lhsT=wb[0:8, ors + r * 128: ors + (r + 1) * 128],
                             rhs=a2h[0:8, :], start=True, stop=True)
            scr = work.tile([128, 512], BF16, tag="scr")
            nc.vector.scalar_tensor_tensor(
                scr[:], hpall[:, cg * 512:(cg + 1) * 512], 1.0, pba[:],
                op0=OP.mult, op1=OP.mult, accum_out=gT[:, cg:cg + 1])

    # dropped-column correction: hp_drop = relu(bf), a2_drop = sig(<relu1,c2>)
    relu1b = rows.tile([128, 1], BF16, tag="relu1b")
    nc.vector.tensor_copy(relu1b[:], relu1[:])
    psd = ps_t.tile([128, 128], F32, tag="tp")
    nc.tensor.matmul(psd[0:1, 0:16], lhsT=relu1b[:], rhs=c2fb[:], start=True,
                     stop=True)
    sdropb = rows.tile([1, 16], BF16, tag="sdropb")
    nc.scalar.activation(sdropb[:], psd[0:1, 0:16], ACT.Sigmoid)
    # relu256 row for the rank-1 correction outer product
    pr = ps_t.tile([128, 128], F32, tag="tp")
    nc.tensor.transpose(pr[0:1, :], relu256[:], identf)
    relu256rb = rows.tile([1, 128], BF16, tag="relu256rb")
    nc.vector.tensor_copy(relu256rb[:], pr[0:1, :])
    pcorr = ps_t.tile([128, 128], F32, tag="tp")
    nc.tensor.matmul(pcorr[:, 0:16], lhsT=relu256rb[:], rhs=sdropb[:],
                     start=True, stop=True)
    gfix = rows.tile([128, 16], F32, tag="gfix")
    nc.vector.tensor_tensor(gfix[:], gT[:], pcorr[:, 0:16], op=OP.subtract)
    if DEBUG:
        nc.sync.dma_start(dbg["gT"].ap(), gfix[:])
    if STAGE < 8:
        _finish_dummy(nc, rows, t_out, ctx)
        return

    # ---- final MLP ---------------------------------------------------------
    gfixb = rows.tile([128, 16], BF16, tag="gfixb")
    nc.vector.tensor_copy(gfixb[:], gfix[:])
    p1 = ps_t.tile([128, 128], F32, tag="tp")
    nc.tensor.matmul(p1[:, 0:NPC], lhsT=W("Wl1a"), rhs=gfixb[:, 0:NPC],
                     start=True, stop=False)
    nc.tensor.matmul(p1[:, 0:NPC], lhsT=W("Wl1b"), rhs=gfixb[:, NPC:2 * NPC],
                     start=False, stop=True)
    o1 = rows.tile([128, NPC], BF16, tag="o1")
    nc.scalar.activation(o1[:], p1[:, 0:NPC], ACT.Relu, bias=Bc("bl1col")[:])
    p2 = ps_t.tile([128, 128], F32, tag="tp")
    nc.tensor.matmul(p2[0:64, 0:NPC], lhsT=W("Wl2"), rhs=o1[:], start=True,
                     stop=True)
    o2 = rows.tile([64, NPC], BF16, tag="o2")
    nc.scalar.activation(o2[:], p2[0:64, 0:NPC], ACT.Relu,
                         bias=Bc("bl2col")[0:64, :])
    p3 = ps_t.tile([128, 128], F32, tag="tp")
    nc.tensor.matmul(p3[0:2, 0:NPC], lhsT=W("Wl3")[0:64, :], rhs=o2[:],
                     start=True, stop=True)
    o3 = rows.tile([2, NPC], F32, tag="o3")
    nc.vector.tensor_scalar(o3[:], p3[0:2, 0:NPC], Bc("bl3col")[0:2, :],
                            None, op0=OP.add)
    nc.sync.dma_start(t_out.ap(), o3[:])
    ctx.close()


def _finish_dummy(nc, rows, t_out, ctx):
    o3 = rows.tile([2, NPC], F32, tag="o3")
    nc.vector.memset(o3[:], 0.0)
    nc.sync.dma_start(t_out.ap(), o3[:])
    ctx.close()


_NC_CACHE = {}


def _get_nc(idxw):
    key = (idxw, STAGE, SUB, DEBUG)
    if key not in _NC_CACHE:
        _NC_CACHE[key] = _build(idxw)
    return _NC_CACHE[key]


def kernel(**inputs):
    in_maps = _host_prep(inputs)
    nc = _get_nc(_GEOM["IDXW"])
    res = run_bass_kernel_spmd(nc, in_maps, core_ids=list(range(NCORES)),
                               trace=TRACE)
    out = np.empty((B, 2), np.float32)
    for c in range(NCORES):
        out[c * NPC:(c + 1) * NPC] = res.results[c]["out"].T
    kernel._last = res
    return out


# revision 43
# speedup vs baseline: 1.0853x; 1.0853x over previous
"""Trainium2 Bass kernel for nn_CAGpool (GNN message passing, CAG pooling).

Sharding: data-parallel over the 64 graph pairs -> 8 pairs (16 component
graphs of 512 nodes) per NeuronCore.  Message passing is dense matmul
against a per-graph 512x512 adjacency built on-device with GPSIMD
local_scatter from host-prepared CSR index lists (index-layout prep only;
all numeric compute happens on device).

v2 restructure vs baseline:
- Both GCN norms folded into C in ONE fused DVE scalar_tensor_tensor pass
  (src norm via per-partition column, dst norm via PE-broadcast row read
  straight from PSUM).
- Attention-pool means ride free on the layer relu stores via the Act
  engine's accum_out; alpha/final pools use single-pass
  tensor_tensor_reduce instead of mult+reduce pairs.
- Pooled-conv XW matmuls + PSUM->SBUF copies are emitted before the
  serial DVE top-k loop so PE/Act overlap it; the sigmoid gate, mask and
  both pooled norms collapse into per-partition scales.
- hp kept in [feat, node] layout; dropped-node bias pollution is removed
  with exact rank-1 corrections instead of extra masking passes.
- Act engine sticks to the {copy,relu,sigmoid,tanh} and {rsqrt} tables
  only (~6 table loads total).
"""

import os
import numpy as np
import ml_dtypes

import concourse.bass as bass
import concourse.tile as tile
from concourse import bacc, mybir
from concourse.bass_utils import run_bass_kernel_spmd

F32 = mybir.dt.float32
BF16 = mybir.dt.bfloat16
I16 = mybir.dt.int16

NCORES = 8
B = 64
NPC = B // NCORES          # graph pairs per core (8)
NCG = 2 * NPC              # component graphs per core (16)
N = 512                    # nodes per component graph
K1 = 256
DEBUG = bool(int(os.environ.get("KERNEL_DEBUG", "0")))
TRACE = bool(int(os.environ.get("KERNEL_TRACE", "0")))
STAGE = int(os.environ.get("KERNEL_STAGE", "9"))
SUB = int(os.environ.get("KERNEL_SUB", "99"))


def _layout(ent):
    offs, off = {}, 0
    for nm, w in ent:
        offs[nm] = (off, w)
        off += w
    return offs, off


WOFF, WF_TOT = _layout(
    [("W1", 128), ("W2", 128), ("W3", 128), ("Wgf", 128)]
    + [(f"Wg{i}", 384) for i in range(3)]
    + [(f"Wal{i}", 768) for i in range(6)]
    + [(f"Wf{i}", 128) for i in range(3)]
    + [("Wl1a", 128), ("Wl1b", 128), ("Wl2", 64), ("Wl3", 2),
       ("cselb", 256), ("rsel", 2048)])
BOFF, BF_TOT = _layout(
    [("bcols", 3), ("bfcol", 1), ("balcol", 6), ("bl1col", 1),
     ("bl2col", 1), ("bl3col", 1), ("identf", 128)])

_GEOM = {}


def _host_prep(inputs):
    """Build per-core input maps. Index-structure prep only."""
    x = np.asarray(inputs["x"], np.float32)

    s_loc, d_loc = {}, {}
    for comp, (sk, dk) in enumerate((("src_c1", "dst_c1"),
                                     ("src_c2", "dst_c2"))):
        base = (np.arange(B) * N)[:, None]
        s_loc[comp] = np.asarray(inputs[sk]).reshape(B, -1) - base
        d_loc[comp] = np.asarray(inputs[dk]).reshape(B, -1) - base

    # Per (graph, comp): unique (s,d) pairs + multi-edge counts + degree.
    per = {}
    degs = {}
    maxw = 2
    for g in range(B):
        for comp in range(2):
            s = s_loc[comp][g]
            d = d_loc[comp][g]
            key = s.astype(np.int64) * N + d.astype(np.int64)
            key = np.concatenate([key, np.arange(N, dtype=np.int64) * (N + 1)])
            uk, cnt = np.unique(key, return_counts=True)
            us = (uk // N).astype(np.int32)
            ud = (uk % N).astype(np.int32)
            per[(g, comp)] = (us, ud, cnt.astype(np.float32))
            # degree (counts by dst, incl +1 self-loop) - integer structure
            degs[(g, comp)] = np.bincount(d, minlength=N).astype(np.float32) + 1.0
            w = np.bincount((us & 127) + 128 * (us >> 8), minlength=256).max()
            maxw = max(maxw, int(w))
    IDXW = (maxw + 1) // 2 * 2
    _GEOM["IDXW"] = IDXW

    in_maps = []
    for c in range(NCORES):
        xT = np.empty((128, NCG * N), ml_dtypes.bfloat16)
        sidx = np.full((128, NCG * 2 * IDXW), -1, np.int16)
        sdat = np.zeros((128, NCG * 2 * IDXW), ml_dtypes.bfloat16)
        degr = np.zeros((16, N), np.float32)
        for comp in range(2):
            for gl in range(NPC):
                g = c * NPC + gl
                cg = comp * NPC + gl
                r0 = g * 2 * N + comp * N
                xT[:, cg * N:(cg + 1) * N] = x[r0:r0 + N].T
                degr[cg] = degs[(g, comp)]
                us, ud, cnt = per[(g, comp)]
                sblk = us >> 7
                p = us & 127
                h = sblk >> 1
                idxval = (sblk - 2 * h) * 512 + ud
                for hh in (0, 1):
                    m = h == hh
                    pp, iv, cv = p[m], idxval[m], cnt[m]
                    order = np.argsort(pp, kind="stable")
                    pp, iv, cv = pp[order], iv[order], cv[order]
                    col = np.zeros(len(pp), np.int64)
                    _, sti, cpn = np.unique(pp, return_index=True,
                                            return_counts=True)
                    for si, cn in zip(sti, cpn):
                        col[si:si + cn] = np.arange(cn)
                    base = (cg * 2 + hh) * IDXW
                    sidx[pp, base + col] = iv.astype(np.int16)
                    sdat[pp, base + col] = cv.astype(ml_dtypes.bfloat16)

        wpack = np.zeros((128, WF_TOT), ml_dtypes.bfloat16)

        def put(nm, arr):
            o, w = WOFF[nm]
            arr = np.asarray(arr, np.float32)
            wpack[: arr.shape[0], o:o + arr.shape[1]] = arr.astype(
                ml_dtypes.bfloat16)

        put("W1", inputs["W1"]); put("W2", inputs["W2"]); put("W3", inputs["W3"])
        put("Wgf", inputs["Wg_fin"])
        for i in range(3):
            put(f"Wg{i}", np.asarray(inputs["Wg_att"])[i * 128:(i + 1) * 128])
        for i in range(6):
            put(f"Wal{i}", np.asarray(inputs["Wal"])[i * 128:(i + 1) * 128])
        for i in range(3):
            put(f"Wf{i}", np.asarray(inputs["Wf"])[i * 128:(i + 1) * 128])
        put("Wl1a", np.asarray(inputs["Wl1"])[:128])
        put("Wl1b", np.asarray(inputs["Wl1"])[128:])
        put("Wl2", inputs["Wl2"])
        put("Wl3", inputs["Wl3"])
        csel = np.zeros((128, 256), np.float32)
        for cg in range(NCG):
            csel[:, cg * 16 + cg] = 1.0
        put("cselb", csel)
        rsel = np.zeros((16, 2048), np.float32)
        for cg in range(16):
            rsel[cg, cg * 128:(cg + 1) * 128] = 1.0
        put("rsel", rsel)

        bpack = np.zeros((128, BF_TOT), np.float32)

        def putb(nm, arr):
            o, w = BOFF[nm]
            arr = np.asarray(arr, np.float32)
            bpack[: arr.shape[0], o:o + arr.shape[1]] = arr

        putb("bcols", np.stack([np.asarray(inputs["b1"]),
                                np.asarray(inputs["b2"]),
                                np.asarray(inputs["b3"])], 1))
        putb("bfcol", np.asarray(inputs["bf"])[:, None])
        putb("balcol", np.asarray(inputs["bal"]).reshape(6, 128).T)
        putb("bl1col", np.asarray(inputs["bl1"])[:, None])
        putb("bl2col", np.asarray(inputs["bl2"])[:, None])
        putb("bl3col", np.asarray(inputs["bl3"])[:, None])
        putb("identf", np.eye(128, dtype=np.float32))

        in_maps.append({"xT": np.ascontiguousarray(xT), "sidx": sidx,
                        "sdat": sdat, "degr": degr,
                        "wpack": np.ascontiguousarray(wpack), "bpack": bpack})
    return in_maps


def _build(idxw):
    nc = bacc.Bacc("TRN2", target_bir_lowering=False, debug=False,
                   num_devices=NCORES)
    tin = {
        "xT": nc.dram_tensor("xT", [128, NCG * N], BF16, kind="ExternalInput"),
        "sidx": nc.dram_tensor("sidx", [128, NCG * 2 * idxw], I16,
                               kind="ExternalInput"),
        "sdat": nc.dram_tensor("sdat", [128, NCG * 2 * idxw], BF16,
                               kind="ExternalInput"),
        "degr": nc.dram_tensor("degr", [16, N], F32, kind="ExternalInput"),
        "wpack": nc.dram_tensor("wpack", [128, WF_TOT], BF16,
                                kind="ExternalInput"),
        "bpack": nc.dram_tensor("bpack", [128, BF_TOT], F32,
                                kind="ExternalInput"),
    }
    t_out = nc.dram_tensor("out", [2, NPC], F32, kind="ExternalOutput")
    dbg = {}
    if DEBUG:
        for nm, shape, dt in (
                ("C", [128, NCG * 2048], BF16), ("deg", [16, N], F32),
                ("xcatT", [128, NCG * 1536], BF16), ("pvT", [128, 48], F32),
                ("scores", [16, N], F32), ("mask", [16, N], F32),
                ("meanT", [128, 48], F32), ("gpT", [128, 48], F32),
                ("deg2", [16, N], F32), ("qrow", [16, N], F32),
                ("hp", [128, NCG * 512], BF16), ("gT", [128, 16], F32),
                ("mean2", [128, 16], F32), ("xwps", [128, NCG * 512], BF16)):
            dbg[nm] = nc.dram_tensor("dbg_" + nm, shape, dt,
                                     kind="ExternalOutput")
    with tile.TileContext(nc, linearize=bool(int(os.environ.get(
            "KERNEL_LINEARIZE", "0")))) as tc:
        _emit(nc, tc, tin, t_out, idxw, dbg)
    nc.compile()
    return nc


def _emit(nc, tc, tin, t_out, idxw, dbg):
    import contextlib
    ctx = contextlib.ExitStack()
    AX = mybir.AxisListType.X
    OP = mybir.AluOpType
    ACT = mybir.ActivationFunctionType

    const = ctx.enter_context(tc.tile_pool(name="const", bufs=1))
    rows = ctx.enter_context(tc.tile_pool(name="rows", bufs=1))
    work = ctx.enter_context(tc.tile_pool(name="work", bufs=2))
    hwork = ctx.enter_context(tc.tile_pool(name="hwork", bufs=2))
    ps_mm = ctx.enter_context(tc.tile_pool(name="ps_mm", bufs=2, space="PSUM"))
    ps_ag = ctx.enter_context(tc.tile_pool(name="ps_ag", bufs=2, space="PSUM"))
    ps_bc = ctx.enter_context(tc.tile_pool(name="ps_bc", bufs=2, space="PSUM"))
    ps_st = ctx.enter_context(tc.tile_pool(name="ps_st", bufs=1, space="PSUM"))
    ps_t = ctx.enter_context(tc.tile_pool(name="ps_t", bufs=1, space="PSUM"))

    wb = const.tile([128, WF_TOT], BF16, tag="wb")
    bp = const.tile([128, BF_TOT], F32, tag="bp")
    xTb = const.tile([128, NCG * N], BF16, tag="xTb")   # reused as hp later
    Call = const.tile([128, NCG * 2048], BF16, tag="Call")
    xcatT = const.tile([128, NCG * 1536], BF16, tag="xcatT")
    rsdcol = const.tile([128, 64], F32, tag="rsdcol")
    msqcol = const.tile([128, 64], F32, tag="msqcol")
    gqcol = const.tile([128, 64], F32, tag="gqcol")
    meanT = const.tile([128, 48], F32, tag="meanT")
    gpT = const.tile([128, 48], F32, tag="gpT")
    cTf = const.tile([128, 48], F32, tag="cTf")
    hsum = const.tile([128, 16], F32, tag="hsum")
    gT = const.tile([128, 16], F32, tag="gT")

    def W(nm):
        o, w = WOFF[nm]
        return wb[:, o:o + w]

    def Bc(nm):
        o, w = BOFF[nm]
        return bp[:, o:o + w]

    nc.sync.dma_start(wb[:], tin["wpack"].ap())
    nc.sync.dma_start(bp[:], tin["bpack"].ap())
    nc.sync.dma_start(xTb[:], tin["xT"].ap())

    identf = Bc("identf")

    def cselb(cg):
        o, _ = WOFF["cselb"]
        return wb[:, o + cg * 16: o + (cg + 1) * 16]

    def rself(cg):
        o, _ = WOFF["rsel"]
        return wb[0:16, o + cg * 128: o + (cg + 1) * 128]

    def bcast_row(row_tile, cg, n):
        """Broadcast row cg of a [16, n] bf16 tile to [128, n] PSUM f32."""
        pb = ps_bc.tile([128, 512], F32, tag="bcast")
        nc.tensor.matmul(pb[:, :n], lhsT=rself(cg), rhs=row_tile[0:16, 0:n],
                         start=True, stop=True)
        return pb

    def t16(row_tile, sblk, out_col, out_tile):
        """Transpose [16,128] f32 row chunk sblk into out_tile[:, 16-col]."""
        pt = ps_t.tile([128, 128], F32, tag="tp")
        nc.tensor.transpose(pt[:, 0:16],
                            row_tile[:, sblk * 128:(sblk + 1) * 128],
                            identf[0:16, 0:16])
        nc.vector.tensor_copy(out_tile[:, out_col:out_col + 16], pt[:, 0:16])

    # ---- degree norm rows (host-shipped integer degree) -------------------
    deg_row = rows.tile([16, N], F32, tag="deg")
    nc.sync.dma_start(deg_row[:], tin["degr"].ap())
    sq1_row = rows.tile([16, N], F32, tag="sq1")
    nc.scalar.activation(sq1_row[:], deg_row[:], ACT.Sqrt)    # table A
    rsd_row = rows.tile([16, N], F32, tag="rsd")
    nc.vector.reciprocal(rsd_row[:], sq1_row[:])
    rsd_rowb = rows.tile([16, N], BF16, tag="rsdb")
    nc.vector.tensor_copy(rsd_rowb[:], rsd_row[:])
    for sblk in range(4):
        t16(rsd_row, sblk, sblk * 16, rsdcol)
    if DEBUG:
        nc.sync.dma_start(dbg["deg"].ap(), deg_row[:])

    # ---- build C (scatter raw counts incl +I) -----------------------------
    with tc.tile_pool(name="edges", bufs=1) as epool:
        sidx = epool.tile([128, NCG * 2 * idxw], I16, tag="sidx")
        sdat = epool.tile([128, NCG * 2 * idxw], BF16, tag="sdat")
        nc.sync.dma_start(sidx[:], tin["sidx"].ap())
        nc.sync.dma_start(sdat[:], tin["sdat"].ap())
        for cg in range(NCG):
            for h in (0, 1):
                b0 = (cg * 2 + h) * idxw
                nc.gpsimd.local_scatter(
                    Call[:, cg * 2048 + h * 1024: cg * 2048 + (h + 1) * 1024],
                    sdat[:, b0:b0 + idxw], sidx[:, b0:b0 + idxw],
                    channels=128, num_elems=1024, num_idxs=idxw)

    # ---- fold BOTH norms into C: Chat = rsd[s] * rsd[d] * C ---------------
    for cg in range(NCG):
        pb = bcast_row(rsd_rowb, cg, N)
        for sblk in range(4):
            sl = Call[:, cg * 2048 + sblk * 512: cg * 2048 + (sblk + 1) * 512]
            nc.vector.scalar_tensor_tensor(
                sl, sl, rsdcol[:, sblk * 16 + cg: sblk * 16 + cg + 1], pb[:],
                op0=OP.mult, op1=OP.mult)
    if DEBUG:
        nc.sync.dma_start(dbg["C"].ap(), Call[:])

    # ---- 3 GCN layers (mean accumulates free via Act accum_out) -----------
    if STAGE < 2:
        _finish_dummy(nc, rows, t_out, ctx)
        return
    for l in range(3):
        wl = W(("W1", "W2", "W3")[l])
        bcol = Bc("bcols")[:, l:l + 1]
        for cg in range(NCG):
            pxw = ps_mm.tile([128, 512], F32, tag="mmw")
            for nt in range(4):
                if l == 0:
                    lhsT = xTb[:, cg * N + nt * 128: cg * N + (nt + 1) * 128]
                else:
                    lhsT = xcatT[:, cg * 1536 + (l - 1) * 512 + nt * 128:
                                 cg * 1536 + (l - 1) * 512 + (nt + 1) * 128]
                nc.tensor.matmul(pxw[:, nt * 128:(nt + 1) * 128], lhsT=lhsT,
                                 rhs=wl, start=True, stop=True)
            xws = work.tile([128, 512], BF16, tag="xws")
            if l == 1:
                nc.vector.tensor_copy(xws[:], pxw[:])
            else:
                nc.scalar.activation(xws[:], pxw[:], ACT.Copy)
            ph = ps_ag.tile([128, 512], F32, tag="agg")
            for sblk in range(4):
                nc.tensor.matmul(
                    ph[:],
                    lhsT=xws[:, sblk * 128:(sblk + 1) * 128],
                    rhs=Call[:, cg * 2048 + sblk * 512:
                             cg * 2048 + (sblk + 1) * 512],
                    start=(sblk == 0), stop=(sblk == 3))
            nc.scalar.activation(
                xcatT[:, cg * 1536 + l * 512: cg * 1536 + (l + 1) * 512],
                ph[:], ACT.Relu, bias=bcol,
                accum_out=meanT[:, l * 16 + cg: l * 16 + cg + 1])
    if DEBUG:
        nc.sync.dma_start(dbg["xcatT"].ap(), xcatT[:])
        nc.sync.dma_start(dbg["meanT"].ap(), meanT[:])

    # ---- attention pool: c = tanh(mean @ Wg) ------------------------------
    if STAGE < 3:
        _finish_dummy(nc, rows, t_out, ctx)
        return
    meanTb = rows.tile([128, 48], BF16, tag="meanTb")
    nc.scalar.activation(meanTb[:], meanT[:], ACT.Copy, scale=1.0 / N)
    for fo in range(3):
        pc = ps_t.tile([128, 128], F32, tag="tp")
        for fi in range(3):
            nc.tensor.matmul(pc[:, 0:16],
                             lhsT=W(f"Wg{fi}")[:, fo * 128:(fo + 1) * 128],
                             rhs=meanTb[:, fi * 16:(fi + 1) * 16],
                             start=(fi == 0), stop=(fi == 2))
        nc.scalar.activation(cTf[:, fo * 16:(fo + 1) * 16], pc[:, 0:16],
                             ACT.Tanh)                     # table B
    if SUB < 2:
        _finish_dummy(nc, rows, t_out, ctx)
        return

    # ---- alpha logits: one [16,512] PSUM pass over xcat -------------------
    ps_al = ps_st.tile([16, N], F32, tag="stat")
    for cg in range(NCG):
        for ch in range(3):
            mlh = work.tile([128, 16], BF16, tag="mlh")
            nc.vector.tensor_scalar(
                mlh[:], cselb(cg), cTf[:, ch * 16 + cg: ch * 16 + cg + 1],
                None, op0=OP.mult)
            nc.tensor.matmul(
                ps_al[:], lhsT=mlh[:],
                rhs=xcatT[:, cg * 1536 + ch * 512: cg * 1536 + (ch + 1) * 512],
                start=(cg == 0 and ch == 0),
                stop=(cg == NCG - 1 and ch == 2))
    alpha_rowb = rows.tile([16, N], BF16, tag="alpha")
    nc.scalar.activation(alpha_rowb[:], ps_al[:], ACT.Sigmoid)
    if SUB < 3:
        _finish_dummy(nc, rows, t_out, ctx)
        return

    # ---- gp = sum_n alpha[n] * xcat[n] via fused TTR ----------------------
    for cg in range(NCG):
        pab = bcast_row(alpha_rowb, cg, N)
        for ch in range(3):
            scr = work.tile([128, 512], BF16, tag="scr")
            nc.vector.scalar_tensor_tensor(
                scr[:],
                xcatT[:, cg * 1536 + ch * 512: cg * 1536 + (ch + 1) * 512],
                1.0, pab[:], op0=OP.mult, op1=OP.mult,
                accum_out=gpT[:, ch * 16 + cg: ch * 16 + cg + 1])
    if DEBUG:
        nc.sync.dma_start(dbg["gpT"].ap(), gpT[:])

    # ---- pv = att_lin(concat pools) ---------------------------------------
    if STAGE < 4:
        _finish_dummy(nc, rows, t_out, ctx)
        return
    gpcatTb = rows.tile([128, 48], BF16, tag="gpcatTb")
    for j in range(6):
        comp, ch = j // 3, j % 3
        nc.vector.tensor_copy(
            gpcatTb[:, j * 8:(j + 1) * 8],
            gpT[:, ch * 16 + comp * 8: ch * 16 + comp * 8 + 8])
    pvTb = rows.tile([128, 48], BF16, tag="pvTb")
    pvTf = rows.tile([128, 48], F32, tag="pvTf")
    for co in range(6):
        pp = ps_t.tile([128, 128], F32, tag="tp")
        for ci in range(6):
            nc.tensor.matmul(pp[:, 0:8],
                             lhsT=W(f"Wal{ci}")[:, co * 128:(co + 1) * 128],
                             rhs=gpcatTb[:, ci * 8:(ci + 1) * 8],
                             start=(ci == 0), stop=(ci == 5))
        nc.vector.tensor_scalar(pvTf[:, co * 8:(co + 1) * 8], pp[:, 0:8],
                                Bc("balcol")[:, co:co + 1], None, op0=OP.add)
        nc.vector.tensor_copy(pvTb[:, co * 8:(co + 1) * 8],
                              pvTf[:, co * 8:(co + 1) * 8])
    if DEBUG:
        nc.sync.dma_start(dbg["pvT"].ap(), pvTf[:])

    # ---- 1/||pv|| ----------------------------------------------------------
    pn = ps_t.tile([128, 128], F32, tag="tp")
    for ci in range(6):
        comp = ci // 3
        mpv = work.tile([128, 16], BF16, tag="mlh")
        nc.vector.memset(mpv[:], 0.0)
        nc.vector.tensor_copy(mpv[:, comp * 8:(comp + 1) * 8],
                              pvTb[:, ci * 8:(ci + 1) * 8])
        nc.tensor.matmul(pn[0:16, 0:16], lhsT=mpv[:], rhs=mpv[:],
                         start=(ci == 0), stop=(ci == 5))
    dd = work.tile([16, 16], F32, tag="dd")
    nc.vector.tensor_tensor(dd[:], pn[0:16, 0:16], identf[0:16, 0:16],
                            op=OP.mult)
    nn = rows.tile([16, 1], F32, tag="nn")
    nc.vector.tensor_reduce(nn[:], dd[:], axis=AX, op=OP.add)
    # ---- scores (raw, unnormalized: topk order is scale-invariant) --------
    ps_sc = ps_st.tile([16, N], F32, tag="stat")
    for cg in range(NCG):
        comp, g = cg // NPC, cg % NPC
        for ci in range(3):
            mlh = work.tile([128, 16], BF16, tag="mlh")
            nc.vector.tensor_scalar(
                mlh[:], cselb(cg),
                pvTf[:, (comp * 3 + ci) * 8 + g:(comp * 3 + ci) * 8 + g + 1],
                None, op0=OP.mult)
            nc.tensor.matmul(
                ps_sc[:], lhsT=mlh[:],
                rhs=xcatT[:, cg * 1536 + ci * 512: cg * 1536 + (ci + 1) * 512],
                start=(cg == 0 and ci == 0),
                stop=(cg == NCG - 1 and ci == 2))
    score_row = rows.tile([16, N], F32, tag="score")
    nc.scalar.activation(score_row[:], ps_sc[:], ACT.Copy)

    # ---- top-256 on raw scores, XWp matmuls interleaved for PE overlap ----
    if STAGE < 5:
        _finish_dummy(nc, rows, t_out, ctx)
        return
    # allocated here so it reuses the SBUF freed by the edges pool
    xwpool = ctx.enter_context(tc.tile_pool(name="xwpool", bufs=1))
    xwps = xwpool.tile([128, NCG * 512], BF16, tag="xwps")

    def emit_xwp(cg):
        pxp = ps_mm.tile([128, 512], F32, tag="mmw")
        for nt in range(4):
            for ci in range(3):
                nc.tensor.matmul(
                    pxp[:, nt * 128:(nt + 1) * 128],
                    lhsT=xcatT[:, cg * 1536 + ci * 512 + nt * 128:
                               cg * 1536 + ci * 512 + (nt + 1) * 128],
                    rhs=W(f"Wf{ci}"), start=(ci == 0), stop=(ci == 2))
        nc.scalar.activation(xwps[:, cg * 512:(cg + 1) * 512], pxp[:],
                             ACT.Copy)

    cur = rows.tile([16, N], F32, tag="cur")
    nc.vector.tensor_copy(cur[:], score_row[:])
    mx = rows.tile([16, 8], F32, tag="mx")
    for r in range(K1 // 8):
        nc.vector.max(out=mx[:], in_=cur[:])
        nc.vector.match_replace(out=cur[:], in_to_replace=mx[:],
                                in_values=cur[:], imm_value=-1e30)
        if r % 2 == 0 and r // 2 < NCG:
            emit_xwp(r // 2)
    # 1/||pv|| computed under the topk window (Act idle there)
    sqn = rows.tile([16, 1], F32, tag="sqn")
    nc.scalar.activation(sqn[:], nn[:], ACT.Sqrt)              # table A
    rsncol = rows.tile([16, 1], F32, tag="rsncol")
    nc.vector.reciprocal(rsncol[:], sqn[:])
    mask_row = rows.tile([16, N], F32, tag="mask")
    nc.vector.tensor_tensor(mask_row[:], score_row[:], cur[:], op=OP.not_equal)
    sig_row = rows.tile([16, N], F32, tag="sig")
    nc.scalar.activation(sig_row[:], score_row[:], ACT.Sigmoid,
                         scale=rsncol[:])                      # table B
    if DEBUG:
        nc.sync.dma_start(dbg["scores"].ap(), score_row[:])
        nc.sync.dma_start(dbg["mask"].ap(), mask_row[:])
    if STAGE < 6:
        _finish_dummy(nc, rows, t_out, ctx)
        return

    # ---- pooled degree: one [16,512] pass over Chat -----------------------
    msq_row = rows.tile([16, N], F32, tag="msq")
    nc.vector.tensor_tensor(msq_row[:], mask_row[:], sq1_row[:], op=OP.mult)
    for sblk in range(4):
        t16(msq_row, sblk, sblk * 16, msqcol)
    ps_d2 = ps_st.tile([16, N], F32, tag="stat")
    for cg in range(NCG):
        for sblk in range(4):
            mlh = work.tile([128, 16], BF16, tag="mlh")
            nc.vector.tensor_scalar(
                mlh[:], cselb(cg),
                msqcol[:, sblk * 16 + cg: sblk * 16 + cg + 1], None,
                op0=OP.mult)
            nc.tensor.matmul(
                ps_d2[:], lhsT=mlh[:],
                rhs=Call[:, cg * 2048 + sblk * 512: cg * 2048 + (sblk + 1) * 512],
                start=(cg == 0 and sblk == 0),
                stop=(cg == NCG - 1 and sblk == 3))
    deg2_row = rows.tile([16, N], F32, tag="deg2")
    nc.vector.tensor_tensor(deg2_row[:], ps_d2[:], sq1_row[:], op=OP.mult)
    nc.vector.scalar_tensor_tensor(deg2_row[:], deg2_row[:], 1.0, mask_row[:],
                                   op0=OP.add, op1=OP.subtract)
    if DEBUG:
        nc.sync.dma_start(dbg["deg2"].ap(), deg2_row[:])
    nc.scalar.activation(deg2_row[:], deg2_row[:], ACT.Sqrt)    # table A
    rsd2_row = rows.tile([16, N], F32, tag="rsd2")
    nc.vector.reciprocal(rsd2_row[:], deg2_row[:])
    q_row = rows.tile([16, N], F32, tag="qrow")
    nc.vector.tensor_tensor(q_row[:], rsd2_row[:], msq_row[:], op=OP.mult)
    qrowb = rows.tile([16, N], BF16, tag="qrowb")
    nc.vector.tensor_copy(qrowb[:], q_row[:])
    gq_row = sig_row  # sigmoid row dead after this product
    nc.vector.tensor_tensor(gq_row[:], sig_row[:], q_row[:], op=OP.mult)
    for sblk in range(4):
        t16(gq_row, sblk, sblk * 16, gqcol)
    if DEBUG:
        nc.sync.dma_start(dbg["qrow"].ap(), q_row[:])

    # ---- pooled conv: scale xwps by gate*rsd2*sq1, agg, relu --------------
    hpall = xTb  # xTb fully consumed by layer 1
    bfcol = Bc("bfcol")[:, 0:1]
    for cg in range(NCG):
        for nt in range(4):
            sl = xwps[:, cg * 512 + nt * 128: cg * 512 + (nt + 1) * 128]
            nc.scalar.activation(sl, sl, ACT.Copy,
                                 scale=gqcol[:, nt * 16 + cg: nt * 16 + cg + 1])
        pm = ps_ag.tile([128, 512], F32, tag="agg")
        for sblk in range(4):
            nc.tensor.matmul(
                pm[:],
                lhsT=xwps[:, cg * 512 + sblk * 128: cg * 512 + (sblk + 1) * 128],
                rhs=Call[:, cg * 2048 + sblk * 512: cg * 2048 + (sblk + 1) * 512],
                start=(sblk == 0), stop=(sblk == 3))
        pbq = bcast_row(qrowb, cg, N)
        qb = work.tile([128, 512], BF16, tag="scr")
        nc.scalar.activation(qb[:], pbq[:], ACT.Copy)
        hpre = hwork.tile([128, 512], F32, tag="hpre")
        nc.vector.tensor_tensor(hpre[:], pm[:], qb[:], op=OP.mult)
        nc.scalar.activation(hpall[:, cg * 512:(cg + 1) * 512], hpre[:],
                             ACT.Relu, bias=bfcol,
                             accum_out=hsum[:, cg:cg + 1])
    if DEBUG:
        nc.sync.dma_start(dbg["hp"].ap(), hpall[:])
        nc.sync.dma_start(dbg["xwps"].ap(), xwps[:])
    if STAGE < 7:
        _finish_dummy(nc, rows, t_out, ctx)
        return

    # ---- final attention pool with exact dropped-column corrections -------
    relu1 = rows.tile([128, 1], F32, tag="relu1")
    nc.scalar.activation(relu1[:], bfcol, ACT.Relu)
    relu256 = rows.tile([128, 1], F32, tag="relu256")
    nc.scalar.activation(relu256[:], bfcol, ACT.Relu, scale=float(N - K1))
    mean2 = rows.tile([128, 16], F32, tag="mean2")
    for cg in range(NCG):
        nc.vector.scalar_tensor_tensor(
            mean2[:, cg:cg + 1], hsum[:, cg:cg + 1], 1.0 / K1, relu1[:],
            op0=OP.mult, op1=OP.subtract)
    # mean2 = hsum/K - (N-K)/K * relu(bf); (N-K)/K = 1 here since K = N/2
    mean2b = rows.tile([128, 16], BF16, tag="mean2b")
    nc.vector.tensor_copy(mean2b[:], mean2[:])
    if DEBUG:
        nc.sync.dma_start(dbg["mean2"].ap(), mean2[:])
    pc2 = ps_t.tile([128, 128], F32, tag="tp")
    nc.tensor.matmul(pc2[:, 0:16], lhsT=W("Wgf"), rhs=mean2b[:], start=True,
                     stop=True)
    c2f = rows.tile([128, 16], F32, tag="c2f")
    nc.scalar.activation(c2f[:], pc2[:, 0:16], ACT.Tanh)        # table B
    c2fb = rows.tile([128, 16], BF16, tag="c2fb")
    nc.vector.tensor_copy(c2fb[:], c2f[:])

    # alpha2 in two half-groups so the g-pool overlaps the second group.
    # Both halves route into PSUM partitions 0-7 (engines cannot read PSUM
    # at base partition 8): selector cols [cg*16+half*8 : +8] put row cg%8.
    ocs, _ = WOFF["cselb"]
    ors, _ = WOFF["rsel"]
    for half in range(2):
        a2h = rows.tile([8, N], BF16, tag=f"a2row{half}")
        ps_a2 = ps_st.tile([16, N], F32, tag="stat")
        for cg in range(half * 8, half * 8 + 8):
            c2m = work.tile([128, 8], BF16, tag="mlh8")
            nc.vector.tensor_scalar(
                c2m[:], wb[:, ocs + cg * 16 + half * 8: ocs + cg * 16 + half * 8 + 8],
                c2f[:, cg:cg + 1], None, op0=OP.mult)
            nc.tensor.matmul(ps_a2[0:8, :], lhsT=c2m[:],
                             rhs=hpall[:, cg * 512:(cg + 1) * 512],
                             start=(cg == half * 8), stop=(cg == half * 8 + 7))
        nc.scalar.activation(a2h[:], ps_a2[0:8, :], ACT.Sigmoid)
        for cg in range(half * 8, half * 8 + 8):
            r = cg % 8
            pba = ps_bc.tile([128, 512], F32, tag="bcast")
            nc.tensor.matmul(pba[:], lhsT=wb[0:8, ors + r * 128: ors + (r + 1) * 128],
                             rhs=a2h[0:8, :], start=True, stop=True)
            scr = work.tile([128, 512], BF16, tag="scr")
            nc.vector.scalar_tensor_tensor(
                scr[:], hpall[:, cg * 512:(cg + 1) * 512], 1.0, pba[:],
                op0=OP.mult, op1=OP.mult, accum_out=gT[:, cg:cg + 1])

    # dropped-column correction: hp_drop = relu(bf), a2_drop = sig(<relu1,c2>)
    relu1b = rows.tile([128, 1], BF16, tag="relu1b")
    nc.vector.tensor_copy(relu1b[:], relu1[:])
    psd = ps_t.tile([128, 128], F32, tag="tp")
    nc.tensor.matmul(psd[0:1, 0:16], lhsT=relu1b[:], rhs=c2fb[:], start=True,
                     stop=True)
    sdropb = rows.tile([1, 16], BF16, tag="sdropb")
    nc.scalar.activation(sdropb[:], psd[0:1, 0:16], ACT.Sigmoid)
    # relu256 row for the rank-1 correction outer product
    pr = ps_t.tile([128, 128], F32, tag="tp")
    nc.tensor.transpose(pr[0:1, :], relu256[:], identf)
    relu256rb = rows.tile([1, 128], BF16, tag="relu256rb")
    nc.vector.tensor_copy(relu256rb[:], pr[0:1, :])
    pcorr = ps_t.tile([128, 128], F32, tag="tp")
    nc.tensor.matmul(pcorr[:, 0:16], lhsT=relu256rb[:], rhs=sdropb[:],
                     start=True, stop=True)
    gfix = rows.tile([128, 16], F32, tag="gfix")
    nc.vector.tensor_tensor(gfix[:], gT[:], pcorr[:, 0:16], op=OP.subtract)
    if DEBUG:
        nc.sync.dma_start(dbg["gT"].ap(), gfix[:])
    if STAGE < 8:
        _finish_dummy(nc, rows, t_out, ctx)
        return

    # ---- final MLP ---------------------------------------------------------
    gfixb = rows.tile([128, 16], BF16, tag="gfixb")
    nc.vector.tensor_copy(gfixb[:], gfix[:])
    p1 = ps_t.tile([128, 128], F32, tag="tp")
    nc.tensor.matmul(p1[:, 0:NPC], lhsT=W("Wl1a"), rhs=gfixb[:, 0:NPC],
                     start=True, stop=False)
    nc.tensor.matmul(p1[:, 0:NPC], lhsT=W("Wl1b"), rhs=gfixb[:, NPC:2 * NPC],
                     start=False, stop=True)
    o1 = rows.tile([128, NPC], BF16, tag="o1")
    nc.scalar.activation(o1[:], p1[:, 0:NPC], ACT.Relu, bias=Bc("bl1col")[:])
    p2 = ps_t.tile([128, 128], F32, tag="tp")
    nc.tensor.matmul(p2[0:64, 0:NPC], lhsT=W("Wl2"), rhs=o1[:], start=True,
                     stop=True)
    o2 = rows.tile([64, NPC], BF16, tag="o2")
    nc.scalar.activation(o2[:], p2[0:64, 0:NPC], ACT.Relu,
                         bias=Bc("bl2col")[0:64, :])
    p3 = ps_t.tile([128, 128], F32, tag="tp")
    nc.tensor.matmul(p3[0:2, 0:NPC], lhsT=W("Wl3")[0:64, :], rhs=o2[:],
                     start=True, stop=True)
    o3 = rows.tile([2, NPC], F32, tag="o3")
    nc.vector.tensor_scalar(o3[:], p3[0:2, 0:NPC], Bc("bl3col")[0:2, :],
                            None, op0=OP.add)
    nc.sync.dma_start(t_out.ap(), o3[:])
    ctx.close()


def _finish_dummy(nc, rows, t_out, ctx):
    o3 = rows.tile([2, NPC], F32, tag="o3")
    nc.vector.memset(o3[:], 0.0)
    nc.sync.dma_start(t_out.ap(), o3[:])
    ctx.close()


_NC_CACHE = {}


def _get_nc(idxw):
    key = (idxw, STAGE, SUB, DEBUG)
    if key not in _NC_CACHE:
        _NC_CACHE[key] = _build(idxw)
    return _NC_CACHE[key]


def kernel(**inputs):
    in_maps = _host_prep(inputs)
    nc = _get_nc(_GEOM["IDXW"])
    res = run_bass_kernel_spmd(nc, in_maps, core_ids=list(range(NCORES)),
                               trace=TRACE)
    out = np.empty((B, 2), np.float32)
    for c in range(NCORES):
        out[c * NPC:(c + 1) * NPC] = res.results[c]["out"].T
    kernel._last = res
    return out
